# revision 1
# baseline (speedup 1.0000x reference)
"""Trainium2 Bass kernel for nn_CustomEPA (channel / cross-covariance attention).

Reference (per batch b, head h; N=4096 tokens, C=768 channels, H=4, dh=192):
    qkv = x @ W_qkv; q,k,v -> (dh, N); q,k L2-normalized over N
    A = softmax(q_hat @ k_hat^T * temp); x_CA = (A @ v)^T; y = x_CA @ W_out + b

Restructuring used here (cuts FLOPs ~2.2x, avoids materializing Q/K/V):
    G   = x^T x                      (contraction over N -> natural layouts)
    S_h = Wq_h^T G Wk_h;  |q_d|^2 = diag(Wq_h^T G Wq_h);  |k_e|^2 likewise
    A_h = softmax(S_h * rq_d * rk_e * temp_h)
    y   = x @ W_eff + b,   W_eff = R^T W_out,   R = stack_h(A_h @ Wv_h^T)

Sharding: batch B=8 across 8 NeuronCores (data parallel, no collectives).
"""

import numpy as np
import ml_dtypes

import concourse.bass as bass
import concourse.tile as tile
from concourse import bacc, mybir
from concourse.bass import ts
from concourse.masks import make_identity
from concourse.bass_utils import run_bass_kernel_spmd

B, N, C, H = 8, 4096, 768, 4
DH = C // H            # 192
P = 128
NT = N // P            # 32
CT = C // P            # 6
BF16 = mybir.dt.bfloat16
FP32 = mybir.dt.float32
MUL = mybir.AluOpType.mult
ADD = mybir.AluOpType.add
AX = mybir.AxisListType.X
EXP = mybir.ActivationFunctionType.Exp
CHUNKS = ((0, P), (P, 64))     # dh=192 split into 128 + 64 partition chunks


def build_kernel(tc):
    nc = tc.nc
    x_nat = nc.dram_tensor("x_nat", [N, C], BF16, kind="ExternalInput").ap()
    xT = nc.dram_tensor("xT", [C, N], BF16, kind="ExternalInput").ap()
    wqk = nc.dram_tensor("wqk", [C, 2 * C], BF16, kind="ExternalInput").ap()
    wvT = nc.dram_tensor("wvT", [C, C], BF16, kind="ExternalInput").ap()
    wout = nc.dram_tensor("wout", [C, C], BF16, kind="ExternalInput").ap()
    bout = nc.dram_tensor("bout", [1, C], FP32, kind="ExternalInput").ap()
    temp = nc.dram_tensor("temp", [1, H], FP32, kind="ExternalInput").ap()
    y = nc.dram_tensor("y", [N, C], FP32, kind="ExternalOutput").ap()

    with tc.tile_pool(name="persist", bufs=1) as persist, \
         tc.tile_pool(name="work", bufs=4) as work, \
         tc.tile_pool(name="small", bufs=4) as small:

        # ---------- constants / weights ----------
        ident = persist.tile([P, P], FP32)
        make_identity(nc, ident)
        ones_row = persist.tile([1, P], FP32)
        nc.vector.memset(ones_row, 1.0)
        ident_bf = persist.tile([P, P], BF16)
        make_identity(nc, ident_bf)

        sb_wqk = persist.tile([P, CT, 2 * C], BF16)
        nc.sync.dma_start(sb_wqk, wqk.rearrange("(t p) c -> p t c", p=P))

        # WvT / W_out per-head-aligned: head h rows [h*192,(h+1)*192) as 128+64
        sb_wvT0 = persist.tile([P, H, C], BF16)
        sb_wvT1 = persist.tile([64, H, C], BF16)
        sb_wout0 = persist.tile([P, H, C], BF16)
        sb_wout1 = persist.tile([64, H, C], BF16)
        for h in range(H):
            nc.sync.dma_start(sb_wvT0[:, h, :], wvT[h * DH: h * DH + P, :])
            nc.sync.dma_start(sb_wvT1[:, h, :], wvT[h * DH + P: (h + 1) * DH, :])
            nc.sync.dma_start(sb_wout0[:, h, :], wout[h * DH: h * DH + P, :])
            nc.sync.dma_start(sb_wout1[:, h, :], wout[h * DH + P: (h + 1) * DH, :])

        # partition-broadcast constants via DMA
        sb_bout = persist.tile([P, C], FP32)
        nc.gpsimd.dma_start(out=sb_bout, in_=bout.to_broadcast([P, C]))
        sb_temp = persist.tile([P, H], FP32)
        nc.gpsimd.dma_start(out=sb_temp, in_=temp.to_broadcast([P, H]))

        sb_G = persist.tile([P, CT, C], BF16)
        sb_GW = persist.tile([P, CT, 2 * C], BF16)
        sb_weff = [persist.tile([P, C], BF16, name=f"weff{i}") for i in range(CT)]
        sb_R0 = persist.tile([P, H, C], BF16)
        sb_R1 = persist.tile([64, H, C], BF16)
        sb_xT = persist.tile([P, CT, N], BF16)
        nc.sync.dma_start(sb_xT, xT.rearrange("(t p) n -> p t n", p=P))

        # ---------- phase 1: G = x^T x  (stream x twice, 6 psum banks) ----------
        with tc.tile_pool(name="pg", bufs=1, space="PSUM") as pg:
            for half in range(2):
                csl = slice(half * 384, half * 384 + 384)
                ms = list(range(3)) if half == 0 else list(range(CT))
                psums = {m: pg.tile([P, 384], FP32, tag=f"g{m}", name=f"psg{m}")
                         for m in ms}
                for kt in range(NT):
                    xg = work.tile([P, C], BF16, tag="xg")
                    nc.sync.dma_start(xg, x_nat[ts(kt, P), :])
                    for m in ms:
                        nc.tensor.matmul(psums[m], lhsT=xg[:, ts(m, P)],
                                         rhs=xg[:, csl],
                                         start=(kt == 0), stop=(kt == NT - 1))
                for m in ms:
                    nc.vector.tensor_copy(out=sb_G[:, m, csl], in_=psums[m])
            # mirror lower-left G[384:768, 0:384] = G[0:384, 384:768]^T
            for a in range(3):          # row-chunk of the mirrored region
                for b in range(3):      # col-chunk
                    ps_gt = pg.tile([P, P], BF16, tag="gt", name="ps_gt", bufs=2)
                    nc.tensor.transpose(
                        ps_gt, sb_G[:, b, 384 + 128 * a: 384 + 128 * a + P],
                        ident_bf)
                    nc.vector.tensor_copy(
                        out=sb_G[:, 3 + a, 128 * b: 128 * b + P], in_=ps_gt)

        # ---------- phase 2: GW = G @ [Wq|Wk]  (G symmetric -> lhsT directly) ----
        with tc.tile_pool(name="pgw", bufs=2, space="PSUM") as pgw:
            for m in range(CT):
                for nh in range(3):
                    csl = slice(nh * 512, nh * 512 + 512)
                    pt = pgw.tile([P, 512], FP32, tag="gw")
                    for kt in range(CT):
                        nc.tensor.matmul(pt, lhsT=sb_G[:, kt, ts(m, P)],
                                         rhs=sb_wqk[:, kt, csl],
                                         start=(kt == 0), stop=(kt == CT - 1))
                    nc.vector.tensor_copy(out=sb_GW[:, m, csl], in_=pt)

        # ---------- phase 3a: per-head S blocks, norms, softmax -> A ----------
        A0s, A1s = [], []
        with tc.tile_pool(name="pz", bufs=1, space="PSUM") as pz:
            for h in range(H):
                q_off, k_off = h * DH, C + h * DH
                ps_qq = [pz.tile([P, P], FP32, tag="qq0", name="psqq0"),
                         pz.tile([64, 64], FP32, tag="qq1", name="psqq1")]
                ps_S = [pz.tile([P, DH], FP32, tag="s0", name="pss0"),
                        pz.tile([64, DH], FP32, tag="s1", name="pss1")]
                ps_kk = [pz.tile([P, P], FP32, tag="kk0", name="pskk0"),
                         pz.tile([64, 64], FP32, tag="kk1", name="pskk1")]
                for mi, (mo, msz) in enumerate(CHUNKS):
                    for kt in range(CT):
                        st, sp = kt == 0, kt == CT - 1
                        lq = sb_wqk[:, kt, q_off + mo: q_off + mo + msz]
                        lk = sb_wqk[:, kt, k_off + mo: k_off + mo + msz]
                        # diag blocks only need their own column range
                        rqd = sb_GW[:, kt, q_off + mo: q_off + mo + msz]
                        rkd = sb_GW[:, kt, k_off + mo: k_off + mo + msz]
                        rk_ap = sb_GW[:, kt, k_off: k_off + DH]
                        nc.tensor.matmul(ps_qq[mi], lhsT=lq, rhs=rqd, start=st, stop=sp)
                        nc.tensor.matmul(ps_S[mi], lhsT=lq, rhs=rk_ap, start=st, stop=sp)
                        nc.tensor.matmul(ps_kk[mi], lhsT=lk, rhs=rkd, start=st, stop=sp)

                # diagonals via identity mask + fused reduce -> [p, 1]
                qq = [small.tile([P, 1], FP32, tag="qq_a", name="qq_a"),
                      small.tile([64, 1], FP32, tag="qq_b", name="qq_b")]
                kk = [small.tile([P, 1], FP32, tag="kk_a", name="kk_a"),
                      small.tile([64, 1], FP32, tag="kk_b", name="kk_b")]
                for mi, (mo, msz) in enumerate(CHUNKS):
                    scr_q = small.tile([P, P], FP32, tag="scr_q", name="scr_q")
                    nc.vector.tensor_tensor(scr_q[:msz, :msz], ps_qq[mi],
                                            ident[:msz, :msz], MUL)
                    nc.vector.reduce_sum(qq[mi], scr_q[:msz, :msz], axis=AX)
                    scr_k = small.tile([P, P], FP32, tag="scr_k", name="scr_k")
                    nc.vector.tensor_tensor(scr_k[:msz, :msz], ps_kk[mi],
                                            ident[:msz, :msz], MUL)
                    nc.vector.reduce_sum(kk[mi], scr_k[:msz, :msz], axis=AX)

                # rq = temp_h / sqrt(qq); rk = 1/sqrt(kk)  (norms >> eps=1e-12)
                rq = [small.tile([P, 1], FP32, tag="rq_a", name="rq_a"),
                      small.tile([64, 1], FP32, tag="rq_b", name="rq_b")]
                rk = [small.tile([P, 1], FP32, tag="rk_a", name="rk_a"),
                      small.tile([64, 1], FP32, tag="rk_b", name="rk_b")]
                for mi, (mo, msz) in enumerate(CHUNKS):
                    nc.scalar.sqrt(rq[mi], qq[mi])
                    nc.vector.reciprocal(rq[mi], rq[mi])
                    nc.vector.tensor_tensor(rq[mi], rq[mi],
                                            sb_temp[:msz, h: h + 1], MUL)
                    nc.scalar.sqrt(rk[mi], kk[mi])
                    nc.vector.reciprocal(rk[mi], rk[mi])

                # rk partition layout -> free layout, then DMA-broadcast
                ps_rk = pz.tile([1, DH], FP32, tag="rkT")
                nc.tensor.transpose(ps_rk[0:1, 0:P], rk[0], ident)
                nc.tensor.transpose(ps_rk[0:1, P:DH], rk[1], ident[0:64, 0:64])
                rk_f = small.tile([1, DH], FP32, tag="rkf")
                nc.vector.tensor_copy(out=rk_f, in_=ps_rk)
                rk_bc = pz.tile([P, DH], FP32, tag="rkbc", name="ps_rkbc")
                nc.tensor.matmul(rk_bc, lhsT=ones_row, rhs=rk_f,
                                 start=True, stop=True)

                # S_hat = S * rq[row] * rk[col]; softmax over free dim
                A0 = small.tile([P, DH], FP32, tag="A0", bufs=4)
                A1 = small.tile([64, DH], FP32, tag="A1", bufs=4)
                A0s.append(A0)
                A1s.append(A1)
                for mi, (mo, msz) in enumerate(CHUNKS):
                    a_dst = A0 if mi == 0 else A1
                    s_sc = small.tile([P, DH], FP32, tag="ssc", name="ssc")[:msz]
                    nc.vector.tensor_scalar_mul(s_sc, ps_S[mi], rq[mi])
                    nc.vector.tensor_tensor(s_sc, s_sc, rk_bc[:msz], MUL)
                    mx = small.tile([P, 1], FP32, tag="mx", name="mx")[:msz]
                    nc.vector.reduce_max(mx, s_sc, axis=AX)
                    nmx = small.tile([P, 1], FP32, tag="nmx", name="nmx")[:msz]
                    nc.vector.tensor_scalar_mul(nmx, mx, -1.0)
                    ssum = small.tile([P, 1], FP32, tag="ssum", name="ssum")[:msz]
                    nc.scalar.activation(out=a_dst, in_=s_sc, func=EXP,
                                         bias=nmx, scale=1.0, accum_out=ssum)
                    rsum = small.tile([P, 1], FP32, tag="rsum", name="rsum")[:msz]
                    nc.vector.reciprocal(rsum, ssum)
                    nc.vector.tensor_scalar_mul(a_dst, a_dst, rsum)

        # ---------- phase 3b: A^T and R_h = (A^T, WvT_h) contraction ----------
        with tc.tile_pool(name="pr", bufs=2, space="PSUM") as pr:
            for h in range(H):
                A0, A1 = A0s[h], A1s[h]
                AT0 = small.tile([P, DH], BF16, tag="AT0", bufs=2)
                AT1 = small.tile([64, DH], BF16, tag="AT1", bufs=2)
                for (do, dsz) in CHUNKS:
                    a_src = A0 if do == 0 else A1
                    for (eo, esz) in CHUNKS:
                        ps_t = pr.tile([P, P], FP32, tag="at", name="ps_at")[:esz, :dsz]
                        nc.tensor.transpose(ps_t, a_src[:, eo: eo + esz],
                                            ident[:dsz, :dsz])
                        dst = AT0 if eo == 0 else AT1
                        nc.vector.tensor_copy(out=dst[:esz, do: do + dsz], in_=ps_t)

                # R[d, c] = sum_e AT[e, d] * WvT_h[e, c]
                for mi, (mo, msz) in enumerate(CHUNKS):
                    ps_r = [pr.tile([P, 384], FP32, tag="r0", name="ps_r0")[:msz],
                            pr.tile([P, 384], FP32, tag="r1", name="ps_r1")[:msz]]
                    for ei, (esrc, wsrc) in enumerate(((AT0, sb_wvT0), (AT1, sb_wvT1))):
                        for nh in range(2):
                            nsl = slice(nh * 384, nh * 384 + 384)
                            nc.tensor.matmul(ps_r[nh],
                                             lhsT=esrc[:, mo: mo + msz],
                                             rhs=wsrc[:, h, nsl],
                                             start=(ei == 0), stop=(ei == 1))
                    dst = sb_R0 if mi == 0 else sb_R1
                    for nh in range(2):
                        nc.vector.tensor_copy(
                            out=dst[:msz, h, nh * 384: nh * 384 + 384],
                            in_=ps_r[nh])

        # ---------- phase 4: W_eff = R^T @ W_out ----------
        with tc.tile_pool(name="pw", bufs=2, space="PSUM") as pw:
            for m in range(CT):
                pt = [pw.tile([P, 384], FP32, tag="w0", name="ps_w0"),
                      pw.tile([P, 384], FP32, tag="w1", name="ps_w1")]
                srcs = [(sb_R0, sb_wout0, P), (sb_R1, sb_wout1, 64)]
                for hi, h in enumerate(range(H)):
                    for ci, (rsrc, wsrc, ksz) in enumerate(srcs):
                        first = hi == 0 and ci == 0
                        last = hi == H - 1 and ci == 1
                        for nh in range(2):
                            nsl = slice(nh * 384, nh * 384 + 384)
                            nc.tensor.matmul(pt[nh],
                                             lhsT=rsrc[:ksz, h, ts(m, P)],
                                             rhs=wsrc[:ksz, h, nsl],
                                             start=first, stop=last)
                for nh in range(2):
                    nc.vector.tensor_copy(
                        out=sb_weff[m][:, nh * 384: nh * 384 + 384], in_=pt[nh])

        # ---------- phase 5: y = x @ W_eff + b_out ----------
        with tc.tile_pool(name="py", bufs=3, space="PSUM") as py:
            for mt in range(NT):
                ps_y = [py.tile([P, 384], FP32, tag="y0", name="ps_y0"),
                        py.tile([P, 384], FP32, tag="y1", name="ps_y1")]
                for kt in range(CT):
                    st, sp = kt == 0, kt == CT - 1
                    for nh in range(2):
                        nc.tensor.matmul(ps_y[nh],
                                         lhsT=sb_xT[:, kt, ts(mt, P)],
                                         rhs=sb_weff[kt][:, nh * 384: nh * 384 + 384],
                                         start=st, stop=sp)
                y_sb = work.tile([P, C], FP32, tag="ysb")
                for nh in range(2):
                    nsl = slice(nh * 384, nh * 384 + 384)
                    nc.vector.tensor_tensor(y_sb[:, nsl], ps_y[nh],
                                            sb_bout[:, nsl], ADD)
                nc.sync.dma_start(y[ts(mt, P), :], y_sb)


_CACHED = {}


def _get_nc():
    if "nc" not in _CACHED:
        nc = bacc.Bacc("TRN2", target_bir_lowering=False, debug=False,
                       enable_asserts=False, num_devices=B)
        with tile.TileContext(nc) as tc:
            build_kernel(tc)
        nc.compile()
        _CACHED["nc"] = nc
    return _CACHED["nc"]


def make_in_maps(x, W_qkv, temperature, W_out, b_out):
    bf16 = ml_dtypes.bfloat16
    x = np.asarray(x, dtype=np.float32)
    W_qkv = np.asarray(W_qkv, dtype=np.float32)
    wqk_h = np.ascontiguousarray(W_qkv[:, : 2 * C].astype(bf16))
    wvT_h = np.ascontiguousarray(W_qkv[:, 2 * C:].T.astype(bf16))
    wout_h = np.ascontiguousarray(np.asarray(W_out, dtype=np.float32).astype(bf16))
    bout_h = np.ascontiguousarray(np.asarray(b_out, dtype=np.float32).reshape(1, C))
    temp_h = np.ascontiguousarray(
        np.asarray(temperature, dtype=np.float32).reshape(1, H))
    in_maps = []
    for b in range(B):
        xb = x[b].astype(bf16)
        in_maps.append({
            "x_nat": np.ascontiguousarray(xb),
            "xT": np.ascontiguousarray(xb.T),
            "wqk": wqk_h, "wvT": wvT_h, "wout": wout_h,
            "bout": bout_h, "temp": temp_h,
        })
    return in_maps


def kernel(x, W_qkv, temperature, W_out, b_out, num_heads, **kw):
    assert int(num_heads) == H and tuple(x.shape) == (B, N, C)
    nc = _get_nc()
    in_maps = make_in_maps(x, W_qkv, temperature, W_out, b_out)
    res = run_bass_kernel_spmd(nc, in_maps, core_ids=list(range(B)), **kw)
    out = np.stack([res.results[b]["y"] for b in range(B)], axis=0)
    return np.ascontiguousarray(out.astype(np.float32))

